# revision 15
# baseline (speedup 1.0000x reference)
"""Trainium2 Bass kernel for nn_AttnBlock (B=4, C=64, H=W=64 self-attention block).

Sharding: 8 cores = (batch b in 0..3) x (query-half in 0..1). Each core
computes attention for 2048 query tokens of one batch element against all
4096 key/value tokens of that element. Weights are replicated.

Layout strategy (per core):
  - x_b as [C=64, N=4096] (channels on partitions)
  - k = WkT.T @ x  -> [64, 4096]
  - q = WqT.T @ xq -> [64, 2048]
  - v in [token, channel] layout [128, 32mt, 65] with a trailing ones
    column (gives the softmax denominator for free in the P.V matmul)
  - scoresT[m, n] = k^T q computed per 128-key-tile into PSUM groups,
    exp()'d by ScalarE directly PSUM->SBUF (scale=1/8, no max subtraction:
    scores are ~N(0, 8^2) so exp(s/8) is far from overflow)
  - htT_aug[65, n] = sum_m v_aug[m, :] pT[m, n]  (row 64 = denominator)
  - out[c, n] = x[c, n] + (Wp @ htT[0:64]) * (1/denominator) broadcast
    (partition-broadcast of the reciprocal row via a K=1 matmul)

All inputs arrive as ONE concatenated DRAM tensor, staged through a single
SBUF tile and fanned out by VectorE copies. This keeps every matmul's
dependencies on one semaphore (the self-loading fp32r LDWEIGHTS can only
encode a limited number of sync waits).
"""

import os
import sys

for _p in ("/opt/trn_rl_repo",):
    if _p not in sys.path:
        sys.path.insert(0, _p)

import numpy as np

import concourse.bacc as bacc
import concourse.bass as bass
import concourse.mybir as mybir
import concourse.tile as tile
from concourse.bass_utils import run_bass_kernel_spmd

B, C, H, W = 4, 64, 64, 64
N = H * W            # 4096 tokens
HALF = N // 2        # 2048 query tokens per core
CHUNK = 512          # query-chunk (psum bank width in fp32)
NCHUNKS = HALF // CHUNK   # 4
MT = N // 128        # 32 key tiles of 128 tokens
XIN = N + HALF + 4 * C    # 6400 columns of concatenated input

F32 = mybir.dt.float32
BF16 = mybir.dt.bfloat16

# matmul operand dtype. fp32/f32r matmuls are "self-loading" (walrus
# generates the LDWEIGHTS internally) and can encode only ONE semaphore
# wait -- Tile routinely needs 2+, so 4-byte matmuls fail codegen with
# "Too many sync wait commands". bf16 keeps LDW/MM as separate
# instructions and streams 1 col/cycle through the PE.
DT_MM = BF16

LAST_RESULTS = None  # test harness can inspect exec_time_ns etc.


def _build_nc():
    nc = bacc.Bacc()

    xin_d = nc.dram_tensor("xin", [C, XIN], F32, kind="ExternalInput")
    out_d = nc.dram_tensor("out", [C, HALF], F32, kind="ExternalOutput")

    EXP = mybir.ActivationFunctionType.Exp
    MUL = mybir.AluOpType.mult
    ADD = mybir.AluOpType.add

    with (
        tile.TileContext(nc) as tc,
        tc.tile_pool(name="main", bufs=1) as mpool,
        tc.tile_pool(name="work", bufs=3) as wpool,
        tc.tile_pool(name="psum", bufs=1, space="PSUM") as ppool,
    ):
        # ---- staging: one DMA, then DVE fan-out ----
        xin = mpool.tile([C, XIN], F32, name="xin")
        nc.sync.dma_start(xin[:], xin_d[:])

        xt = mpool.tile([C, N], DT_MM, name="xt")
        xq = mpool.tile([C, HALF], DT_MM, name="xq_sb")
        wcat = mpool.tile([C, 4, C], DT_MM, name="wcat")
        nc.vector.tensor_copy(xt[:], xin[:, 0:N])
        nc.vector.tensor_copy(xq[:], xin[:, N : N + HALF])
        nc.vector.tensor_copy(
            wcat[:], xin[:, N + HALF :].rearrange("c (g d) -> c g d", g=4)
        )
        wq, wk, wv, wp = (wcat[:, i, :] for i in range(4))

        ones1 = mpool.tile([1, C], DT_MM, name="ones1")
        nc.vector.memset(ones1[:], 1.0)

        q_sb = mpool.tile([C, HALF], DT_MM, name="q_sb")
        k_sb = mpool.tile([C, N], DT_MM, name="k_sb")
        v_sb = mpool.tile([128, MT, C + 1], DT_MM, name="v_sb")  # +ones col
        pT = mpool.tile([128, MT, CHUNK], DT_MM, name="pT")
        nc.vector.memset(v_sb[:, :, C : C + 1], 1.0)

        # ---- k / q / v projections ----
        # k: [64, 4096] in two 4-bank psum rounds
        for r in range(2):
            ps_k = ppool.tile([128, 4, CHUNK], F32, name="ps_k", tag="gA")
            for j in range(4):
                ch = r * 4 + j
                nc.tensor.matmul(
                    ps_k[:C, j, :], wk,
                    xt[:, ch * CHUNK : (ch + 1) * CHUNK],
                    start=True, stop=True,
                )
            nc.vector.tensor_copy(
                k_sb[:, r * 4 * CHUNK : (r + 1) * 4 * CHUNK].rearrange(
                    "c (a b) -> c a b", a=4
                ),
                ps_k[:C],
            )

        # q: [64, 2048] in one 4-bank round
        ps_q = ppool.tile([128, 4, CHUNK], F32, name="ps_q", tag="gA")
        for j in range(4):
            nc.tensor.matmul(
                ps_q[:C, j, :], wq,
                xq[:, j * CHUNK : (j + 1) * CHUNK],
                start=True, stop=True,
            )
        nc.vector.tensor_copy(
            q_sb[:].rearrange("c (a b) -> c a b", a=4), ps_q[:C]
        )

        # v in [token, channel] layout: lhsT = x chunk (stationary),
        # rhs = WvT -> psum [128 tokens, 64 ch]
        ps_v = ppool.tile([128, 4, 8, C], F32, name="ps_v", tag="gA")
        for mt in range(MT):
            nc.tensor.matmul(
                ps_v[:, mt // 8, mt % 8, :],
                xt[:, mt * 128 : (mt + 1) * 128], wv,
                start=True, stop=True,
            )
        nc.vector.tensor_copy(
            v_sb[:, :, :C].rearrange("p (a b) c -> p a b c", a=4), ps_v[:]
        )

        # ---- attention over query chunks ----
        # psum plan per chunk: scores in groups of 4 banks (tag gA) and
        # 3 banks (tag gB) alternating (5xA + 4xB covers 32 key tiles);
        # 1 bank (tag mix) rotates for PV accum / broadcast / projection.
        groups = []
        mt0 = 0
        for g in range(9):
            gs = 4 if g % 2 == 0 else 3
            groups.append((mt0, gs))
            mt0 += gs
        assert mt0 == MT

        for ch in range(NCHUNKS):
            qc = q_sb[:, ch * CHUNK : (ch + 1) * CHUNK]

            for (m0, gs) in groups:
                tag = "gA" if gs == 4 else "gB"
                ps_s = ppool.tile([128, gs, CHUNK], F32, name="ps_s", tag=tag)
                for j in range(gs):
                    mt = m0 + j
                    nc.tensor.matmul(
                        ps_s[:, j, :],
                        k_sb[:, mt * 128 : (mt + 1) * 128], qc,
                        start=True, stop=True,
                    )
                # exp((k^T q) / sqrt(C)) straight PSUM -> SBUF
                nc.scalar.activation(
                    pT[:, m0 : m0 + gs, :], ps_s[:], EXP, bias=0.0, scale=0.125
                )

            # P.V with ones-augmented v: row C of the result is the
            # softmax denominator per query.
            ps_pv = ppool.tile([C + 1, CHUNK], F32, name="ps_pv", tag="mix")
            for mt in range(MT):
                nc.tensor.matmul(
                    ps_pv[:], v_sb[:, mt, :], pT[:, mt, :],
                    start=(mt == 0), stop=(mt == MT - 1),
                )
            htT = wpool.tile([C, CHUNK], DT_MM, name="htT", tag="htT")
            nc.vector.tensor_copy(htT[:], ps_pv[:C])
            denom = wpool.tile([1, CHUNK], F32, name="denom", tag="denom")
            nc.vector.tensor_copy(denom[:], ps_pv[C : C + 1, :])

            recip = wpool.tile([1, CHUNK], DT_MM, name="recip", tag="recip")
            with nc.allow_low_precision(
                reason="1/denom feeds a bf16 broadcast matmul; rounding "
                "the reciprocal to bf16 is the intended cost"
            ):
                nc.vector.reciprocal(recip[:], denom[:])

            # broadcast 1/denominator across 64 partitions via K=1 matmul
            ps_b = ppool.tile([C, CHUNK], F32, name="ps_b", tag="mix")
            nc.tensor.matmul(ps_b[:], ones1[:], recip[:], start=True, stop=True)
            rb = wpool.tile([C, CHUNK], F32, name="rb", tag="rb")
            nc.vector.tensor_copy(rb[:], ps_b[:])

            # project the un-normalized ht; the 1/denominator scale commutes
            # with the (linear) projection and is applied at the end.
            ps_o = ppool.tile([C, CHUNK], F32, name="ps_o", tag="mix")
            nc.tensor.matmul(ps_o[:], wp, htT[:], start=True, stop=True)

            out_sb = wpool.tile([C, CHUNK], F32, name="out_sb", tag="out_sb")
            nc.vector.tensor_tensor(out_sb[:], ps_o[:], rb[:], MUL)
            nc.vector.tensor_tensor(
                out_sb[:], out_sb[:],
                xin[:, N + ch * CHUNK : N + (ch + 1) * CHUNK], ADD
            )
            nc.sync.dma_start(out_d[:, ch * CHUNK : (ch + 1) * CHUNK], out_sb[:])

    nc.compile()
    return nc


_NC = None


def _get_nc():
    global _NC
    if _NC is None:
        _NC = _build_nc()
    return _NC


def kernel(x, Wq, Wk, Wv, Wp):
    global LAST_RESULTS
    nc = _get_nc()

    x = np.ascontiguousarray(x, dtype=np.float32)
    wall = np.stack(
        [Wq.T, Wk.T, Wv.T, Wp.T], axis=1
    ).reshape(C, 4 * C).astype(np.float32)  # [c_in, 4*c_out]

    in_maps = []
    for core in range(8):
        b, half = core >> 1, core & 1
        xb = x[b].reshape(C, N)
        xin = np.concatenate(
            [xb, xb[:, half * HALF : (half + 1) * HALF], wall], axis=1
        )
        in_maps.append({"xin": np.ascontiguousarray(xin)})

    res = run_bass_kernel_spmd(nc, in_maps, list(range(8)))
    LAST_RESULTS = res

    y = np.empty((B, C, N), dtype=np.float32)
    for core in range(8):
        b, half = core >> 1, core & 1
        y[b, :, half * HALF : (half + 1) * HALF] = res.results[core]["out"]
    return y.reshape(B, C, H, W)


# revision 22
# speedup vs baseline: 1.0295x; 1.0295x over previous
"""Trainium2 Bass kernel for nn_AttnBlock (B=4, C=64, H=W=64 self-attention block).

Sharding: 8 cores = (batch b in 0..3) x (query-half in 0..1). Each core
computes attention for 2048 query tokens of one batch element against all
4096 key/value tokens of that element. Weights are replicated.

Layout strategy (per core):
  - x_b as [C=64, N=4096] (channels on partitions)
  - k = WkT.T @ x  -> [64, 4096]
  - q = WqT.T @ xq -> [64, 2048]
  - v in [token, channel] layout [128, 32mt, 65] with a trailing ones
    column (gives the softmax denominator for free in the P.V matmul)
  - scoresT[m, n] = k^T q computed per 128-key-tile into PSUM groups,
    exp()'d by ScalarE directly PSUM->SBUF (scale=1/8, no max subtraction:
    scores are ~N(0, 8^2) so exp(s/8) is far from overflow)
  - htT_aug[65, n] = sum_m v_aug[m, :] pT[m, n]  (row 64 = denominator)
  - out[c, n] = x[c, n] + (Wp @ htT[0:64]) * (1/denominator) broadcast
    (partition-broadcast of the reciprocal row via a K=1 matmul)

Inputs arrive pre-converted to bf16 (matmul operands: x | x-query-half |
weights, one concatenated tensor) plus the fp32 query-half of x for the
exact residual add. This removes all staging copies and halves input DMA.
"""

import os
import sys

for _p in ("/opt/trn_rl_repo",):
    if _p not in sys.path:
        sys.path.insert(0, _p)

import numpy as np

import concourse.bacc as bacc
import concourse.bass as bass
import concourse.mybir as mybir
import concourse.tile as tile
from concourse.bass_utils import run_bass_kernel_spmd

B, C, H, W = 4, 64, 64, 64
N = H * W            # 4096 tokens
HALF = N // 2        # 2048 query tokens per core
CHUNK = 512          # query-chunk (psum bank width in fp32)
NCHUNKS = HALF // CHUNK   # 4
MT = N // 128        # 32 key tiles of 128 tokens
# packed [128, XIN2] input: per partition-half -> [x-half | xq-half | weights]
XIN2 = N // 2 + HALF // 2 + 4 * C   # 3328 columns per partition row

F32 = mybir.dt.float32
BF16 = mybir.dt.bfloat16

# matmul operand dtype. fp32/f32r matmuls are "self-loading" (walrus
# generates the LDWEIGHTS internally) and can encode only ONE semaphore
# wait -- Tile routinely needs 2+, so 4-byte matmuls fail codegen with
# "Too many sync wait commands". bf16 keeps LDW/MM as separate
# instructions and streams 1 col/cycle through the PE.
DT_MM = BF16

LAST_RESULTS = None  # test harness can inspect exec_time_ns etc.


def _build_nc():
    nc = bacc.Bacc()

    # Packed 128-partition inputs for full DMA bandwidth:
    #   xin128[p, :]: for p<64 (channel c=p) columns hold
    #     [x chunks 0-3 | xq chunks 0-1 | wq wk] and for p>=64 (c=p-64)
    #     [x chunks 4-7 | xq chunks 2-3 | wv wp].
    xin_d = nc.dram_tensor("xin", [128, XIN2], BF16, kind="ExternalInput")
    xres_d = nc.dram_tensor("xres", [C, HALF], F32, kind="ExternalInput")
    out_d = nc.dram_tensor("out", [C, HALF], F32, kind="ExternalOutput")

    EXP = mybir.ActivationFunctionType.Exp
    MUL = mybir.AluOpType.mult
    ADD = mybir.AluOpType.add

    with (
        tile.TileContext(nc) as tc,
        tc.tile_pool(name="main", bufs=1) as mpool,
        tc.tile_pool(name="work", bufs=3) as wpool,
        tc.tile_pool(name="psum", bufs=1, space="PSUM") as ppool,
    ):
        xin = mpool.tile([128, XIN2], BF16, name="xin")
        nc.sync.dma_start(xin[:], xin_d[:])
        xres = mpool.tile([C, HALF], F32, name="xres")
        nc.sync.dma_start(xres[:], xres_d[:])

        def xt_cols(c0, w):
            """x[:, c0:c0+w] as a [64, w] AP (w must stay in one 2048-col half)."""
            half, off = divmod(c0, N // 2)
            assert off + w <= N // 2
            return xin[64 * half : 64 * half + 64, off : off + w]

        def xq_cols(c0, w):
            half, off = divmod(c0, HALF // 2)
            assert off + w <= HALF // 2
            base = N // 2
            return xin[64 * half : 64 * half + 64, base + off : base + off + w]

        def w_g(g, half=0):
            # weights are replicated on both partition halves so lhsT can
            # match the rhs's base partition (PE rows = SBUF partitions)
            base = N // 2 + HALF // 2
            return xin[64 * half : 64 * half + 64, base + g * C : base + (g + 1) * C]

        def xres_cols(c0, w):
            return xres[:, c0 : c0 + w]

        wq, wk, wv, wp = w_g(0), w_g(1), w_g(2), w_g(3)

        ones1 = mpool.tile([1, C], DT_MM, name="ones1")
        nc.vector.memset(ones1[:], 1.0)

        q_sb = mpool.tile([C, HALF], DT_MM, name="q_sb")
        k_sb = mpool.tile([C, N], DT_MM, name="k_sb")
        v_sb = mpool.tile([128, MT, C + 1], DT_MM, name="v_sb")  # +ones col
        pT = mpool.tile([128, MT, CHUNK], DT_MM, name="pT")
        nc.vector.memset(v_sb[:, :, C : C + 1], 1.0)

        # ---- q / k / v projections ----
        # PSUM tags: s = [128,3,512] double-buffered scores groups (6 banks),
        # pv = PV accumulator (1 bank), tail = broadcast/projection (1 bank).
        ps_q = ppool.tile([128, 3, CHUNK], F32, name="ps_q", tag="s", bufs=2)
        for j in range(3):
            nc.tensor.matmul(
                ps_q[:C, j, :], w_g(0, j // 2), xq_cols(j * CHUNK, CHUNK),
                start=True, stop=True,
            )
        nc.vector.tensor_copy(
            q_sb[:, 0 : 3 * CHUNK].rearrange("c (a b) -> c a b", a=3), ps_q[:C]
        )

        ps_k = ppool.tile([128, 3, CHUNK], F32, name="ps_k", tag="s", bufs=2)
        for j in range(3):
            nc.tensor.matmul(
                ps_k[:C, j, :], w_g(1, 0), xt_cols(j * CHUNK, CHUNK),
                start=True, stop=True,
            )
        nc.vector.tensor_copy(
            k_sb[:, 0 : 3 * CHUNK].rearrange("c (a b) -> c a b", a=3), ps_k[:C]
        )

        ps_q2 = ppool.tile([128, CHUNK], F32, name="ps_q2", tag="pv")
        nc.tensor.matmul(
            ps_q2[:C, :], w_g(0, 1), xq_cols(3 * CHUNK, CHUNK),
            start=True, stop=True,
        )
        nc.vector.tensor_copy(q_sb[:, 3 * CHUNK :], ps_q2[:C])

        ps_k2 = ppool.tile([128, 3, CHUNK], F32, name="ps_k2", tag="s", bufs=2)
        for j in range(3):
            ch = 3 + j
            nc.tensor.matmul(
                ps_k2[:C, j, :], w_g(1, (ch >= 4)), xt_cols(ch * CHUNK, CHUNK),
                start=True, stop=True,
            )
        nc.vector.tensor_copy(
            k_sb[:, 3 * CHUNK : 6 * CHUNK].rearrange("c (a b) -> c a b", a=3),
            ps_k2[:C],
        )

        ps_k3 = ppool.tile([128, CHUNK], F32, name="ps_k3", tag="tail2")
        nc.tensor.matmul(
            ps_k3[:C, :], w_g(1, 1), xt_cols(6 * CHUNK, CHUNK),
            start=True, stop=True,
        )
        nc.vector.tensor_copy(k_sb[:, 6 * CHUNK : 7 * CHUNK], ps_k3[:C])
        ps_k4 = ppool.tile([128, CHUNK], F32, name="ps_k4", tag="tail2")
        nc.tensor.matmul(
            ps_k4[:C, :], w_g(1, 1), xt_cols(7 * CHUNK, CHUNK),
            start=True, stop=True,
        )
        nc.vector.tensor_copy(k_sb[:, 7 * CHUNK :], ps_k4[:C])

        # v in [token, channel] layout: lhsT = x 128-token chunk, rhs = WvT
        ps_v = ppool.tile([128, 3, 8, C], F32, name="ps_v", tag="s", bufs=2)
        for mt in range(24):
            nc.tensor.matmul(
                ps_v[:, mt // 8, mt % 8, :],
                xt_cols(mt * 128, 128), w_g(2, mt // 16),
                start=True, stop=True,
            )
        nc.vector.tensor_copy(
            v_sb[:, 0:24, :C].rearrange("p (a b) c -> p a b c", a=3), ps_v[:]
        )
        ps_v2 = ppool.tile([128, 8, C], F32, name="ps_v2", tag="pv")
        for mt in range(24, MT):
            nc.tensor.matmul(
                ps_v2[:, mt - 24, :], xt_cols(mt * 128, 128), w_g(2, 1),
                start=True, stop=True,
            )
        nc.vector.tensor_copy(v_sb[:, 24:MT, :C], ps_v2[:])

        # ---- attention over query chunks (software-pipelined) ----
        # scores+exp for chunk ch overlap P.V for chunk ch-1: PV matmuls are
        # interleaved between score groups on the PE queue so ScalarE (the
        # bottleneck: 8.4M exps) never starves. One uniform score tag with
        # bufs=2 rotates globally -- no pipeline drain at chunk boundaries.
        groups = []
        mt0 = 0
        while mt0 < MT:
            gs = min(3, MT - mt0)
            groups.append((mt0, gs))
            mt0 += gs

        state = {}

        def emit_tail(ch):
            """normalize, project, residual, store for chunk ch."""
            pv = state.pop("pv")
            htT = wpool.tile([C, CHUNK], DT_MM, name="htT", tag="htT")
            nc.vector.tensor_copy(htT[:], pv[:C])
            denom = wpool.tile([1, CHUNK], F32, name="denom", tag="denom")
            nc.vector.tensor_copy(denom[:], pv[C : C + 1, :])

            recip = wpool.tile([1, CHUNK], DT_MM, name="recip", tag="recip")
            with nc.allow_low_precision(
                reason="1/denom feeds a bf16 broadcast matmul; rounding "
                "the reciprocal to bf16 is the intended cost"
            ):
                nc.vector.reciprocal(recip[:], denom[:])

            # broadcast 1/denominator across 64 partitions via K=1 matmul
            ps_b = ppool.tile([C, CHUNK], F32, name="ps_b", tag="tail2")
            nc.tensor.matmul(ps_b[:], ones1[:], recip[:], start=True, stop=True)
            rb = wpool.tile([C, CHUNK], F32, name="rb", tag="rb")
            nc.vector.tensor_copy(rb[:], ps_b[:])

            # project the un-normalized ht; the 1/denominator scale commutes
            # with the (linear) projection and is applied at the end.
            ps_o = ppool.tile([C, CHUNK], F32, name="ps_o", tag="tail2")
            nc.tensor.matmul(ps_o[:], w_g(3, 0), htT[:], start=True, stop=True)

            out_sb = wpool.tile([C, CHUNK], F32, name="out_sb", tag="out_sb")
            nc.vector.tensor_tensor(out_sb[:], ps_o[:], rb[:], MUL)
            nc.vector.tensor_tensor(
                out_sb[:], out_sb[:], xres_cols(ch * CHUNK, CHUNK), ADD
            )
            nc.sync.dma_start(out_d[:, ch * CHUNK : (ch + 1) * CHUNK], out_sb[:])

        for ph in range(NCHUNKS + 1):
            for gi, (m0, gs) in enumerate(groups):
                if ph > 0:
                    # P.V slice for the previous chunk (same mts whose pT
                    # this group's exp will overwrite right after)
                    if gi == 0:
                        state["pv"] = ppool.tile(
                            [C + 1, CHUNK], F32, name="ps_pv", tag="pv"
                        )
                    for mt in range(m0, m0 + gs):
                        nc.tensor.matmul(
                            state["pv"][:], v_sb[:, mt, :], pT[:, mt, :],
                            start=(mt == 0), stop=(mt == MT - 1),
                        )
                if ph < NCHUNKS:
                    ps_s = ppool.tile([128, 3, CHUNK], F32, name="ps_s", tag="s", bufs=2)
                    for j in range(gs):
                        mt = m0 + j
                        nc.tensor.matmul(
                            ps_s[:, j, :],
                            k_sb[:, mt * 128 : (mt + 1) * 128],
                            q_sb[:, ph * CHUNK : (ph + 1) * CHUNK],
                            start=True, stop=True,
                        )
                    # exp((k^T q) / sqrt(C)) straight PSUM -> SBUF
                    nc.scalar.activation(
                        pT[:, m0 : m0 + gs, :], ps_s[:, :gs, :], EXP,
                        bias=0.0, scale=0.125,
                    )
            if ph > 0:
                emit_tail(ph - 1)

    nc.compile()
    return nc


_NC = None


def _get_nc():
    global _NC
    if _NC is None:
        _NC = _build_nc()
    return _NC


def kernel(x, Wq, Wk, Wv, Wp):
    global LAST_RESULTS
    nc = _get_nc()

    import ml_dtypes
    x = np.ascontiguousarray(x, dtype=np.float32)
    wall = np.concatenate(
        [Wq.T, Wk.T, Wv.T, Wp.T], axis=1
    ).astype(np.float32)  # [c_in, 4*c_out] = [64, 256]

    in_maps = []
    for core in range(8):
        b, half = core >> 1, core & 1
        xb = x[b].reshape(C, N)
        xh = xb[:, half * HALF : (half + 1) * HALF]
        lo = np.concatenate([xb[:, : N // 2], xh[:, : HALF // 2], wall], axis=1)
        hi = np.concatenate([xb[:, N // 2 :], xh[:, HALF // 2 :], wall], axis=1)
        xin = np.concatenate([lo, hi], axis=0).astype(ml_dtypes.bfloat16)
        in_maps.append({
            "xin": np.ascontiguousarray(xin),
            "xres": np.ascontiguousarray(xh),
        })

    res = run_bass_kernel_spmd(nc, in_maps, list(range(8)))
    LAST_RESULTS = res

    y = np.empty((B, C, N), dtype=np.float32)
    for core in range(8):
        b, half = core >> 1, core & 1
        y[b, :, half * HALF : (half + 1) * HALF] = res.results[core]["out"]
    return y.reshape(B, C, H, W)
